# revision 7
# baseline (speedup 1.0000x reference)
"""DistMaps kernel for Trainium2 (Bass/Tile), SPMD over 8 NeuronCores.

Problem: out[b, 0, z, y, x] = 1.0 if min_p ((z-pz)^2 + (y-py)^2 + (x-px)^2) <= 25
for the 24 points p of batch b, else 0.0.  (x input is only used for its shape.)

Strategy
--------
The output is a union of radius-5 balls around 24 points per batch.  Shard the
volume over D (96 = 8 cores x 12 z-slices); every core computes its z-slab for
both batches, so no collective is needed.

Per z-slice the output plane is a union of 24 disks.  A disk at (py, px) with
radius^2 t is, row by row, an x-interval:  ind[y', x] = (x-px)^2 <= t - (y'-py)^2.
We enumerate k-rows = (point p, row offset j in -4..5), giving K = 240 rows per
batch covering every integer y with |y - py| <= 5 (10 rows suffice for
non-integer py).  Then

  plane[y, x] = sum_k onehot[k, y] * ind_z[k, x]   (TensorE matmul, bf16->f32)
  out = plane > 0                                  (ScalarE Sign / DVE is_gt)

where onehot[k, y] = (y == floor(py_k) + j_k) is z-invariant and ind_z is one
cheap DVE tensor_scalar (dx2 <= t[k, z]) per (slice, row-group).  Rows whose
threshold is negative (point too far in z or y) contribute all-zero rows
automatically, so the program is fully static and identical on all cores.
"""

import numpy as np

B = 2
D, H, W = 96, 160, 160
P = 24          # points per batch element
J = 10          # k-rows per point, j = -4..5
K = P * J       # 240 k-rows per batch
NCORES = 8
DLOC = D // NCORES  # 12 z-slices per core
ZG = 3          # z-slices per matmul group (3*160 = 480 <= 512 fp32 PSUM bank)
NZG = DLOC // ZG
RG = (128, K - 128)  # k-row groups (partition tiles): 128 + 112
R2 = 25.0

_prog_cache = {}


def _build_program():
    from contextlib import ExitStack

    import concourse.mybir as mybir
    import concourse.tile as tile
    from concourse import bacc

    f32 = mybir.dt.float32
    bf16 = mybir.dt.bfloat16
    op = mybir.AluOpType

    nc = bacc.Bacc(trn_type="TRN2")

    pts_d = nc.dram_tensor("pts", [B, K, 4], f32, kind="ExternalInput")
    zgb_d = nc.dram_tensor("zgridb", [128, DLOC], f32, kind="ExternalInput")
    xgb_d = nc.dram_tensor("xgridb", [128, W], f32, kind="ExternalInput")
    # out[b, y, zloc, x]: y-major so one (b, zgroup, ytile) store is a single
    # DMA with 480 contiguous elements per partition.
    out_d = nc.dram_tensor("out", [B, H, DLOC, W], f32, kind="ExternalOutput")

    with tile.TileContext(nc) as tc, ExitStack() as ctx:
        const = ctx.enter_context(tc.tile_pool(name="const", bufs=1))
        setup = ctx.enter_context(tc.tile_pool(name="setup", bufs=1))
        rhsp = ctx.enter_context(tc.tile_pool(name="rhsp", bufs=4))
        # bufs=16: one output-staging slot per (b, zgroup) iteration, so no
        # store DMA ever waits on a previous DMA for slot reuse (walrus
        # rejects DMAs with more than one sync-wait).
        outp = ctx.enter_context(tc.tile_pool(name="outp", bufs=16))
        psump = ctx.enter_context(tc.tile_pool(name="psump", bufs=4, space="PSUM"))

        xgb = const.tile([128, W], f32, name="xgb")
        nc.gpsimd.dma_start(out=xgb[:, :], in_=xgb_d[:, :])
        zgb = const.tile([128, DLOC], f32, name="zgb")
        nc.gpsimd.dma_start(out=zgb[:, :], in_=zgb_d[:, :])

        onehot = {}
        dx2 = {}
        tt = {}
        for b in range(B):
            for g, rg in enumerate(RG):
                r0 = g * 128
                sfx = f"{b}_{g}"
                psb = setup.tile([128, 4], f32, name=f"pts_{sfx}")
                nc.gpsimd.dma_start(out=psb[:rg, :], in_=pts_d[b, r0 : r0 + rg, :])
                pz = psb[:rg, 0:1]
                py = psb[:rg, 1:2]
                px = psb[:rg, 2:3]
                jc = psb[:rg, 3:4]

                col = setup.tile([128, 4], f32, name=f"col_{sfx}")
                yfl = col[:rg, 0:1]
                ypr = col[:rg, 1:2]
                dyj = col[:rg, 2:3]
                cc = col[:rg, 3:4]
                # floor(py) = round(py - 0.5) via the fp32 magic-number trick
                # (round-to-nearest falls out of adding 1.5*2^23).  A tie at
                # integer py may give floor-1, which only shifts the 10-row
                # window; rows stay self-consistent so the result is the same.
                MAGIC = float(12582912.0)  # 1.5 * 2**23
                nc.vector.tensor_scalar(
                    out=yfl, in0=py, scalar1=-0.5, scalar2=MAGIC, op0=op.add,
                    op1=op.add,
                )
                nc.vector.tensor_scalar(
                    out=yfl, in0=yfl, scalar1=MAGIC, scalar2=None, op0=op.subtract
                )
                nc.vector.tensor_tensor(out=ypr, in0=yfl, in1=jc, op=op.add)
                nc.vector.tensor_tensor(out=dyj, in0=ypr, in1=py, op=op.subtract)
                nc.vector.tensor_tensor(out=dyj, in0=dyj, in1=dyj, op=op.mult)
                # cc = 25 - dyj2
                nc.vector.tensor_scalar(
                    out=cc, in0=dyj, scalar1=-1.0, scalar2=R2, op0=op.mult, op1=op.add
                )

                # t[k, zloc] = cc - (z - pz)^2
                dzt = setup.tile([128, DLOC], f32, name=f"dzt_{sfx}")
                nc.vector.tensor_scalar(
                    out=dzt[:rg, :], in0=zgb[:rg, :], scalar1=pz, scalar2=None,
                    op0=op.subtract,
                )
                nc.vector.tensor_tensor(
                    out=dzt[:rg, :], in0=dzt[:rg, :], in1=dzt[:rg, :], op=op.mult
                )
                t_g = setup.tile([128, DLOC], f32, name=f"tt_{sfx}")
                nc.vector.tensor_scalar(
                    out=t_g[:rg, :], in0=dzt[:rg, :], scalar1=cc, scalar2=-1.0,
                    op0=op.subtract, op1=op.mult,
                )

                # dx2[k, x] = (x - px)^2
                dxt = setup.tile([128, W], f32, name=f"dxt_{sfx}")
                nc.vector.tensor_scalar(
                    out=dxt[:rg, :], in0=xgb[:rg, :], scalar1=px, scalar2=None,
                    op0=op.subtract,
                )
                d2 = setup.tile([128, W], f32, name=f"dx2_{sfx}")
                nc.vector.tensor_tensor(
                    out=d2[:rg, :], in0=dxt[:rg, :], in1=dxt[:rg, :], op=op.mult
                )

                # onehot[k, y] = (y == floor(py) + j), bf16 for the matmul
                oh = setup.tile([128, W], bf16, name=f"oh_{sfx}")
                nc.vector.tensor_scalar(
                    out=oh[:rg, :], in0=xgb[:rg, :], scalar1=ypr, scalar2=None,
                    op0=op.is_equal,
                )

                onehot[b, g] = oh
                dx2[b, g] = d2
                tt[b, g] = t_g

        NW = ZG * W  # 480
        for b in range(B):
            for zg in range(NZG):
                rhs = []
                for g, rg in enumerate(RG):
                    rhs_g = rhsp.tile([128, NW], bf16, name=f"rhs{g}", tag=f"rhs{g}")
                    for zi in range(ZG):
                        z = zg * ZG + zi
                        nc.vector.tensor_scalar(
                            out=rhs_g[:rg, zi * W : (zi + 1) * W],
                            in0=dx2[b, g][:rg, :],
                            scalar1=tt[b, g][:rg, z : z + 1],
                            scalar2=None,
                            op0=op.is_le,
                        )
                    rhs.append(rhs_g)

                ps0 = psump.tile([128, NW], f32, name="ps0", tag="ps0")
                ps1 = psump.tile([32, NW], f32, name="ps1", tag="ps1")
                for g, rg in enumerate(RG):
                    nc.tensor.matmul(
                        out=ps0[:, :], lhsT=onehot[b, g][:rg, 0:128],
                        rhs=rhs[g][:rg, :], start=(g == 0), stop=(g == 1),
                    )
                for g, rg in enumerate(RG):
                    nc.tensor.matmul(
                        out=ps1[:, :], lhsT=onehot[b, g][:rg, 128:160],
                        rhs=rhs[g][:rg, :], start=(g == 0), stop=(g == 1),
                    )

                ob0 = outp.tile([128, NW], f32, name="ob0", tag="ob0")
                ob1 = outp.tile([32, NW], f32, name="ob1", tag="ob1")
                nc.scalar.activation(
                    out=ob0[:, :], in_=ps0[:, :],
                    func=mybir.ActivationFunctionType.Sign,
                )
                nc.vector.tensor_scalar(
                    out=ob1[:, :], in0=ps1[:, :], scalar1=0.0, scalar2=None,
                    op0=op.is_gt,
                )

                zl = zg * ZG
                nc.gpsimd.dma_start(
                    out=out_d[b, 0:128, zl : zl + ZG, :],
                    in_=ob0[:, :].rearrange("p (z x) -> p z x", z=ZG),
                )
                nc.gpsimd.dma_start(
                    out=out_d[b, 128:160, zl : zl + ZG, :],
                    in_=ob1[:, :].rearrange("p (z x) -> p z x", z=ZG),
                )

    nc.finalize()
    return nc


def _get_program():
    if "nc" not in _prog_cache:
        _prog_cache["nc"] = _build_program()
    return _prog_cache["nc"]


def kernel(x: np.ndarray, coords: np.ndarray) -> np.ndarray:
    from concourse.bass_utils import run_bass_kernel_spmd

    assert x.shape == (B, 4, D, H, W)
    coords = np.ascontiguousarray(coords, dtype=np.float32)
    assert coords.shape == (B * P, 3)

    # Per-k-row point table: each point repeated J times with its row offset.
    jcol = np.tile(np.arange(-4, 6, dtype=np.float32), P)  # (K,)
    pts = np.empty((B, K, 4), dtype=np.float32)
    for b in range(B):
        cb = coords[b * P : (b + 1) * P]
        pts[b, :, 0:3] = np.repeat(cb, J, axis=0)
        pts[b, :, 3] = jcol

    xgridb = np.ascontiguousarray(
        np.broadcast_to(np.arange(W, dtype=np.float32), (128, W))
    )

    nc = _get_program()
    in_maps = []
    for core in range(NCORES):
        zbase = core * DLOC
        zgridb = np.ascontiguousarray(
            np.broadcast_to(
                np.arange(zbase, zbase + DLOC, dtype=np.float32), (128, DLOC)
            )
        )
        in_maps.append({"pts": pts, "zgridb": zgridb, "xgridb": xgridb})

    res = run_bass_kernel_spmd(nc, in_maps, list(range(NCORES)))

    full = np.empty((B, 1, D, H, W), dtype=np.float32)
    for core in range(NCORES):
        o = res.results[core]["out"]  # [B, H, DLOC, W]
        full[:, 0, core * DLOC : (core + 1) * DLOC] = o.transpose(0, 2, 1, 3)
    return full
